# revision 1
# baseline (speedup 1.0000x reference)
"""DFNet (3-directional Mamba + 1x1 proj + MLP) Trainium2 Bass kernel.

Self-contained: builds the 8-core SPMD Bass program, shards the full inputs
host-side, runs via run_bass_kernel_spmd, gathers the full output.
"""
import sys
for _p in ("/opt/trn_rl_repo", "/root/.axon_site/_ro/trn_rl_repo"):
    if _p not in sys.path:
        sys.path.insert(0, _p)

"""Inlined walrus workarounds for this walrus build, which rejects any instruction carrying
more than ONE sem-wait ("Too many sync wait commands").

1. Patch TileContext._add_instruction: split extra waits onto preceding
   single-wait NoOp instructions on the same engine.
2. Patch TileContext._drain_and_barrier: emit the end-of-kernel drain as a
   chain of single-wait drains.
"""
import concourse.tile as tile_mod
from concourse import mybir
from concourse.vector_clock import ScopedClock, VectorClock

_orig_add_instruction = tile_mod.TileContext._add_instruction
_split_counter = [0]


def _patched_add_instruction(self, inst):
    si = inst.sync_info
    if si is not None and inst.engine != mybir.EngineType.Unassigned:
        waits = list(si.on_wait or [])
        if len(waits) > 1:
            for w in waits[:-1]:
                _split_counter[0] += 1
                nop = mybir.InstNoOp(name=f"{inst.name}-ws{_split_counter[0]}")
                nop.engine = inst.engine
                nop.sync_info = mybir.SyncInfo(on_wait=[w], on_update=[])
                _orig_add_instruction(self, nop)
            inst.sync_info = mybir.SyncInfo(
                on_wait=[waits[-1]], on_update=list(si.on_update or [])
            )
    _orig_add_instruction(self, inst)


def _patched_drain_and_barrier(self, tick_clock, wait_clock):
    gc = tick_clock.global_clock
    n = len(gc)
    for i in range(n):
        t = gc[i]
        if t > 0:
            single = VectorClock([0] * n)
            single.require_at_least(i, t)
            d = self.nc.sync.drain()
            wait_clock.add_sem_waits(d.ins, ScopedClock({None: single}))
    self.nc.sync.drain()

    self.nc.all_engine_barrier()
    assert self.sems is not None
    popped = self.nc._tile_sem_poison_stack.pop()
    assert popped is self._sem_poison
    self.nc.clear_and_free_semaphores(list(self.sems.allocated().values()))
    self.nc.all_engine_barrier()


def apply():
    tile_mod.TileContext._add_instruction = _patched_add_instruction
    tile_mod.TileContext._drain_and_barrier = _patched_drain_and_barrier

apply()  # patch TileContext before building

"""DFNet (3-direction Mamba block + 1x1 proj + MLP) Trainium2 kernel.

Distribution over 8 cores (SPMD-safe: every core runs the same program; all
core-dependent values arrive as per-core input DATA, all cross-core movement
uses AllToAll slot structure so no AP depends on the core id):
  P1: token-parallel (each core: one l-chunk of L/8 per direction).
  A2A#1: reshard (delta, xc, B/C rows) to scan owners, per-quarter slots.
  P2: 24 quarters = (3 dir x 2 dhalf) units x 4 n-groups, 3 per core;
      tensor_tensor_scan along t, one strip per state index n.
  A2A#2: per-quarter partial-y, column-sliced back to token owners.
  P3: token-parallel: sum co-owner partials, gating, fused out_proj+proj,
      residual, LN, MLP.
"""
import numpy as np
from contextlib import ExitStack

import concourse.bass as bass
import concourse.tile as tile
from concourse import mybir
from concourse.tile import add_dep_helper

FP32 = mybir.dt.float32
BF16 = mybir.dt.bfloat16
FP8 = mybir.dt.float8e4
AF = mybir.ActivationFunctionType
ALU = mybir.AluOpType


class Dims:
    def __init__(self, C=128, E=16, n_cores=8):
        self.C = C
        self.E = E
        self.L = E ** 3
        self.NDIR = 3
        self.D_INNER = 2 * C            # 256
        self.NST = 16
        self.DT_RANK = (C + 15) // 16   # 8
        self.D_CONV = 4
        self.n_cores = n_cores
        self.LC = self.L // n_cores
        self.NDH = self.D_INNER // 128  # 2
        self.NQ_TOT = self.NDIR * self.NDH * 4          # 24
        self.QPC = self.NQ_TOT // n_cores               # 3
        assert self.L % n_cores == 0 and self.NQ_TOT % n_cores == 0
        # A2A#1 slot layout (rows x LC), bf16 rows:
        #  per quarter j: [j*80, +8) dt rows (bf16);
        #  [+8, +72): 64 rows holding xc fp8 bytes (row b: xc row b in bytes
        #  [0:LC), xc row 64+b in bytes [LC:2LC));
        #  then per quarter j: 8 bc rows (bf16) at QPC*80 + j*8.
        self.SLOT_ROWS = self.QPC * 80 + self.QPC * 8  # 264
        self.YR = self.QPC * 128                        # A2A#2 slot rows (384)

    def quarters_of_core(self, c):
        out = []
        for j in range(self.QPC):
            qi = c * self.QPC + j
            u, k = qi // 4, qi % 4
            out.append((u // self.NDH, u % self.NDH, 4 * k))
        return out


def ref_forward_np(x, w):
    """Numpy replica of reference.py for arbitrary E (float64 truth)."""
    C = x.shape[1]; E = x.shape[2]; L = E ** 3
    D_INNER = 2 * C; NST = 16; DT_RANK = (C + 15) // 16; D_CONV = 4
    x = x.astype(np.float64)
    g = {k: v.astype(np.float64) for k, v in w.items() if k != "x"}

    def ln_cf(t, wt, bt, eps=1e-6):
        u = t.mean(1, keepdims=True)
        s = ((t - u) ** 2).mean(1, keepdims=True)
        return wt[None, :, None, None, None] * ((t - u) / np.sqrt(s + eps)) \
            + bt[None, :, None, None, None]

    x5 = x.reshape(1, C, E, E, E)
    x1 = ln_cf(x5, g["ln_w"], g["ln_b"])
    xd = x1.reshape(1, C, L)
    xh = x1.transpose(0, 1, 3, 4, 2).reshape(1, C, L)
    xw = x1.transpose(0, 1, 4, 2, 3).reshape(1, C, L)
    seq = np.stack([xd, xh, xw], 0).reshape(3, C, L).swapaxes(1, 2)
    u_ = seq.mean(-1, keepdims=True)
    s_ = ((seq - u_) ** 2).mean(-1, keepdims=True)
    seq = (seq - u_) / np.sqrt(s_ + 1e-5) * g["mnorm_w"] + g["mnorm_b"]
    xz = seq @ g["in_proj_w"].T
    xr, z = xz[..., :D_INNER], xz[..., D_INNER:]
    xp = np.pad(xr, ((0, 0), (D_CONV - 1, 0), (0, 0)))
    xc = sum(g["conv_w"][:, k] * xp[:, k:k + L, :] for k in range(D_CONV)) + g["conv_b"]
    xc = xc * (1 / (1 + np.exp(-xc)))
    x_dbl = xc @ g["x_proj_w"].T
    dt = x_dbl[..., :DT_RANK]
    Bm = x_dbl[..., DT_RANK:DT_RANK + NST]
    Cm = x_dbl[..., DT_RANK + NST:]
    da = dt @ g["dt_proj_w"].T + g["dt_proj_b"]
    delta = np.log1p(np.exp(da))
    A = -np.exp(g["A_log"])
    N, Ln, d = xc.shape
    h = np.zeros((N, d, NST))
    ys = np.zeros((N, Ln, d))
    for t in range(Ln):
        dA = np.exp(delta[:, t, :, None] * A[None])
        dBu = delta[:, t, :, None] * Bm[:, t, None, :] * xc[:, t, :, None]
        h = dA * h + dBu
        ys[:, t] = np.einsum("bdn,bn->bd", h, Cm[:, t])
    y = ys + xc * g["D_param"]
    y = y * (z * (1 / (1 + np.exp(-z))))
    y = y @ g["out_proj_w"].T
    cat = y.swapaxes(1, 2).reshape(3, C, E, E, E)[None].transpose(1, 0, 2, 3, 4, 5)
    cat = cat.reshape(1, 3 * C, E, E, E)
    out1 = np.einsum("bkdhw,ok->bodhw", cat, g["proj_w"]) \
        + g["proj_b"][None, :, None, None, None]
    out_res = x5 + out1
    hh = ln_cf(out_res, g["ln_w"], g["ln_b"])
    hh = np.einsum("bcdhw,oc->bodhw", hh, g["fc1_w"]) + g["fc1_b"][None, :, None, None, None]
    from scipy.special import erf
    hh = hh * 0.5 * (1 + erf(hh / np.sqrt(2)))
    hh = np.einsum("bcdhw,oc->bodhw", hh, g["fc2_w"]) + g["fc2_b"][None, :, None, None, None]
    return (hh + out_res).astype(np.float32)


def perms(E):
    A = np.arange(E ** 3).reshape(E, E, E)
    return [A.ravel(), A.transpose(1, 2, 0).ravel(), A.transpose(2, 0, 1).ravel()]


def host_prep(dm: Dims, inputs):
    w = {k: np.asarray(v, np.float32) for k, v in inputs.items()}
    C, E, L, LC = dm.C, dm.E, dm.L, dm.LC
    x2d = w["x"].reshape(C, L)
    Xg = np.stack([x2d[:, p] for p in perms(E)], 0)

    A_vals = -np.exp(w["A_log"])[0, :]
    Wcomb = np.stack([w["proj_w"][:, g * C:(g + 1) * C] @ w["out_proj_w"]
                      for g in range(3)], 0)
    WcombT = np.ascontiguousarray(Wcomb.transpose(0, 2, 1))  # (3, D_INNER, C)

    shared = {
        "w_inT": np.ascontiguousarray(w["in_proj_w"].T),
        "xprojT": np.ascontiguousarray(w["x_proj_w"].T),  # cast to bf16 below
        "conv_w": w["conv_w"],
        "conv_b": np.ascontiguousarray(w["conv_b"][:, None]),
        "D_col": np.ascontiguousarray(w["D_param"][:, None]),
        "WcombT": WcombT,
        "proj_b": np.ascontiguousarray(w["proj_b"][:, None]),
        "fc1T": np.ascontiguousarray(w["fc1_w"].T),
        "fc2T": np.ascontiguousarray(w["fc2_w"].T),
        "fc1_b": np.ascontiguousarray(w["fc1_b"][:, None]),
        "fc2_b": np.ascontiguousarray(w["fc2_b"][:, None]),
        "ln_w": np.ascontiguousarray(w["ln_w"][:, None]),
        "ln_b": np.ascontiguousarray(w["ln_b"][:, None]),
        "mnorm_w": np.ascontiguousarray(w["mnorm_w"][:, None]),
        "mnorm_b": np.ascontiguousarray(w["mnorm_b"][:, None]),
    }
    import ml_dtypes
    shared["xprojT"] = shared["xprojT"].astype(ml_dtypes.bfloat16)
    shared["ident"] = np.eye(128, dtype=ml_dtypes.bfloat16)
    in_maps = []
    for c in range(dm.n_cores):
        lo = c * LC
        xs = np.zeros((3, C, LC + 3), np.float32)
        xs[:, :, 3:] = Xg[:, :, lo:lo + LC]
        if c > 0:
            xs[:, :, :3] = Xg[:, :, lo - 3:lo]
        m = dict(shared)
        m["xs"] = xs
        m["halo_mask"] = np.full((1, 3), 0.0 if c == 0 else 1.0, np.float32)
        m["x_slice"] = np.ascontiguousarray(x2d[:, lo:lo + LC])
        # per-core scan scales: col j*4+nn = -A[n] for this core's quarters
        na = np.zeros((128, dm.QPC * 4), np.float32)
        for j, (g, dh, n0) in enumerate(dm.quarters_of_core(c)):
            for nn in range(4):
                na[:, j * 4 + nn] = -A_vals[n0 + nn]
        m["negA"] = na
        dtT = w["dt_proj_w"].T  # (RK, DI)
        dq = np.zeros((dm.DT_RANK, dm.QPC * 128), np.float32)
        nb = np.zeros((dm.QPC * 128, 1), np.float32)
        for j, (g, dh, n0) in enumerate(dm.quarters_of_core(c)):
            dq[:, j * 128:(j + 1) * 128] = dtT[:, dh * 128:(dh + 1) * 128]
            nb[j * 128:(j + 1) * 128, 0] = -w["dt_proj_b"][dh * 128:(dh + 1) * 128]
        m["dtprojT_q"] = dq.astype(ml_dtypes.bfloat16)
        m["negdtb_q"] = nb
        in_maps.append(m)
    return in_maps


def build_program(dm: Dims):
    C, E, L, LC = dm.C, dm.E, dm.L, dm.LC
    DI, RK, NST = dm.D_INNER, dm.DT_RANK, dm.NST
    NC, QPC = dm.n_cores, dm.QPC
    NOT = 2 * DI // 128            # 4 o-tiles in xz
    NDT = DI // 128                # 2 d-tiles

    q_table = [dm.quarters_of_core(c) for c in range(NC)]
    # co-owners of each unit (g,dh): list of (src_core, local_quarter)
    co_own = {}
    for s in range(NC):
        for j, (g, dh, n0) in enumerate(q_table[s]):
            co_own.setdefault((g, dh), []).append((s, j))

    nc = bass.Bass()
    def inp(name, shape, dt=FP32):
        return nc.dram_tensor(name, list(shape), dt, kind="ExternalInput")

    xs = inp("xs", (3, C, LC + 3))
    halo_mask = inp("halo_mask", (1, 3))
    x_slice = inp("x_slice", (C, LC))
    w_inT = inp("w_inT", (C, 2 * DI))
    xprojT = inp("xprojT", (DI, RK + 2 * NST), BF16)
    ident = inp("ident", (128, 128), BF16)
    dtprojT_q = inp("dtprojT_q", (RK, QPC * 128), BF16)
    negdtb_q = inp("negdtb_q", (QPC * 128, 1))
    conv_w = inp("conv_w", (DI, 4))
    conv_b = inp("conv_b", (DI, 1))
    negA = inp("negA", (128, QPC * 4))
    D_col = inp("D_col", (DI, 1))
    WcombT = inp("WcombT", (3, DI, C))
    proj_b = inp("proj_b", (C, 1))
    fc1T = inp("fc1T", (C, 4 * C))
    fc2T = inp("fc2T", (4 * C, C))
    fc1_b = inp("fc1_b", (4 * C, 1))
    fc2_b = inp("fc2_b", (C, 1))
    ln_w = inp("ln_w", (C, 1)); ln_b = inp("ln_b", (C, 1))
    mnorm_w = inp("mnorm_w", (C, 1)); mnorm_b = inp("mnorm_b", (C, 1))

    out_slice = nc.dram_tensor("out_slice", [C, LC], FP32, kind="ExternalOutput")

    a2a_in = [nc.dram_tensor("a2a_in0", [NC, 80, LC], BF16),
              nc.dram_tensor("a2a_in1", [NC, 160, LC], BF16)]
    a2a_out = [nc.dram_tensor("a2a_out0", [NC, 80, LC], BF16),
               nc.dram_tensor("a2a_out1", [NC, 160, LC], BF16)]
    ya_in = [nc.dram_tensor(f"ya_in{j}", [NC, 128, LC], FP8) for j in range(QPC)]
    ya_out = [nc.dram_tensor(f"ya_out{j}", [NC, 128, LC], FP8) for j in range(QPC)]

    with ExitStack() as ctx:
        tc = ctx.enter_context(tile.TileContext(nc))
        consts = ctx.enter_context(tc.tile_pool(name="consts", bufs=1))
        keep = ctx.enter_context(tc.tile_pool(name="keep", bufs=1))
        p1_cm = tc.tile_pool(name="p1", bufs=2)
        p1 = p1_cm.__enter__()
        p1ps_cm = tc.tile_pool(name="p1ps", bufs=1, space="PSUM")
        p1ps = p1ps_cm.__enter__()

        pack_writes = [[], []]
        ya_ccs = []

        # ---- constants ----
        def load2d(t, r, k, dt=FP32, tag=None):
            """load (r,k) DRAM into list of (<=128, k) tiles"""
            tiles = []
            for i in range((r + 127) // 128):
                n = min(128, r - i * 128)
                s = consts.tile([n, k], dt, tag=(tag or t.name) + str(i), name=(tag or t.name) + str(i))
                nc.sync.dma_start(out=s, in_=t[i * 128:i * 128 + n, :])
                tiles.append(s)
            return tiles

        w_inT_sb = load2d(w_inT, C, 2 * DI)[0]
        xprojT_t = load2d(xprojT, DI, RK + 2 * NST, BF16)
        ident_sb = load2d(ident, 128, 128, BF16)[0]
        dtprojTq_sb = load2d(dtprojT_q, RK, QPC * 128, BF16)[0]
        negdtbq_t = load2d(negdtb_q, QPC * 128, 1)
        convw_t = load2d(conv_w, DI, 4)
        convb_t = load2d(conv_b, DI, 1)
        negA_sb = load2d(negA, 128, QPC * 4)[0]
        Dcol_t = load2d(D_col, DI, 1)
        projb_sb = load2d(proj_b, C, 1)[0]
        fc1T_sb = load2d(fc1T, C, 4 * C)[0]
        fc2T_t = load2d(fc2T, 4 * C, C)
        fc1b_t = load2d(fc1_b, 4 * C, 1)
        fc2b_sb = load2d(fc2_b, C, 1)[0]
        lnw_sb = load2d(ln_w, C, 1)[0]; lnb_sb = load2d(ln_b, C, 1)[0]
        mw_sb = load2d(mnorm_w, C, 1)[0]; mb_sb = load2d(mnorm_b, C, 1)[0]
        Wct = {}
        for g in range(3):
            for dh in range(dm.NDH):
                s = consts.tile([128, C], FP32, tag=f"wc{g}{dh}", name=f"wc{g}{dh}")
                nc.sync.dma_start(out=s, in_=WcombT[g, dh * 128:(dh + 1) * 128, :])
                Wct[(g, dh)] = s
        mask_sb = consts.tile([128, 3], FP32)
        nc.sync.dma_start(out=mask_sb, in_=halo_mask[:, :].to_broadcast((128, 3)))
        ones_sb = consts.tile([C, 1], FP32)
        nc.vector.memset(ones_sb, 1.0 / C)
        onesr_sb = consts.tile([1, 128], FP32)
        nc.vector.memset(onesr_sb, 1.0)

        z_keep = [[keep.tile([128, LC], BF16, tag=f"zk{g}_{d}", name=f"zk{g}_{d}") for d in range(NDT)]
                  for g in range(3)]
        xc_keep = [[keep.tile([128, LC], BF16, tag=f"xck{g}_{d}", name=f"xck{g}_{d}") for d in range(NDT)]
                   for g in range(3)]

        # ========== P1 ==========
        def part_norm(pool, pspool, x_sb, ncols, wcol, bcol, eps, sfx):
            def mm_chunks(out_ps, lhsT, rhs_sb, rcols):
                for c0 in range(0, rcols, 512):
                    cw = min(512, rcols - c0)
                    nc.tensor.matmul(out_ps[:, c0:c0 + cw], lhsT,
                                     rhs_sb[:, c0:c0 + cw], start=True, stop=True)
            scr = pspool.tile([1, ncols], FP32, tag="scrP", name="scrP", bufs=1)
            mm_chunks(scr, ones_sb[:, :], x_sb, ncols)
            musq = pool.tile([1, ncols], FP32, tag="musq" + sfx)
            nc.scalar.activation(musq[:, :], scr[:, :], AF.Square)
            mu_sb = pool.tile([1, ncols], FP32, tag="musb" + sfx)
            nc.scalar.copy(mu_sb[:, :], scr[:, :])
            sq = pool.tile([C, ncols], FP32, tag="sq" + sfx)
            nc.scalar.activation(sq[:, :], x_sb[:, :ncols], AF.Square)
            scr2 = pspool.tile([1, ncols], FP32, tag="scrP", name="scrP2", bufs=1)
            mm_chunks(scr2, ones_sb[:, :], sq, ncols)
            var = pool.tile([1, ncols], FP32, tag="var" + sfx)
            nc.vector.scalar_tensor_tensor(var[:, :], scr2[:, :], 1.0, musq[:, :],
                                           ALU.mult, ALU.subtract)
            nc.vector.tensor_scalar(var[:, :], var[:, :], eps, None, ALU.add)
            vre = pool.tile([1, ncols], FP32, tag="vre" + sfx)
            nc.vector.reciprocal(vre[:, :], var[:, :])
            rstd = pool.tile([1, ncols], FP32, tag="rstd" + sfx)
            nc.scalar.activation(rstd[:, :], vre[:, :], AF.Sqrt)
            bc = pspool.tile([C, ncols], FP32, tag="bcP", name="bcP", bufs=1)
            mm_chunks(bc, onesr_sb[:1, :C], mu_sb[:1, :], ncols)
            t1 = pool.tile([C, ncols], FP32, tag="n1" + sfx)
            nc.vector.tensor_tensor(t1[:, :], x_sb[:, :ncols], bc[:, :], ALU.subtract)
            bc2 = pspool.tile([C, ncols], FP32, tag="bcP", name="bcP2", bufs=1)
            mm_chunks(bc2, onesr_sb[:1, :C], rstd[:1, :], ncols)
            t2 = pool.tile([C, ncols], FP32, tag="n2" + sfx)
            nc.vector.tensor_tensor(t2[:, :], t1[:, :], bc2[:, :], ALU.mult)
            t3 = pool.tile([C, ncols], FP32, tag="n3" + sfx)
            nc.vector.tensor_scalar(t3[:, :], t2[:, :], wcol[:, :], bcol[:, :],
                                    ALU.mult, ALU.add)
            return t3

        for g in range(3):
            ncols = LC + 3
            x_sb = p1.tile([C, ncols], FP32, tag="x_in")
            nc.sync.dma_start(out=x_sb, in_=xs[g, :, :])
            xn1 = part_norm(p1, p1ps, x_sb, ncols, lnw_sb, lnb_sb, 1e-6, "a")
            xn = part_norm(p1, p1ps, xn1, ncols, mw_sb, mb_sb, 1e-5, "b")
            xr_sb = []
            for ot in range(NOT):
                psw = 512 * ((ncols + 511) // 512)
                ps = p1ps.tile([128, psw], FP32, tag="xzps", name="xzps", bufs=1)
                n1 = min(512, ncols)
                nc.tensor.matmul(ps[:, 0:n1], w_inT_sb[:, ot * 128:(ot + 1) * 128],
                                 xn[:, 0:n1], start=True, stop=True)
                if ncols > 512:
                    nc.tensor.matmul(ps[:, 512:ncols],
                                     w_inT_sb[:, ot * 128:(ot + 1) * 128],
                                     xn[:, 512:ncols], start=True, stop=True)
                if ot < NDT:
                    t = p1.tile([128, ncols], FP32, tag="xr")
                    nc.scalar.copy(t[:, :], ps[:, 0:ncols])
                    nc.vector.tensor_tensor(t[:, 0:3], t[:, 0:3], mask_sb[:, :], ALU.mult)
                    xr_sb.append(t)
                else:
                    zt = z_keep[g][ot - NDT]
                    nc.vector.tensor_copy(zt[:, :], ps[:, 3:ncols])
            xc_sb = []
            for di in range(NDT):
                acc = p1.tile([128, LC], FP32, tag="convacc")
                nc.vector.tensor_scalar(acc[:, :], xr_sb[di][:, 0:LC],
                                        convw_t[di][:, 0:1], None, ALU.mult)
                for k in range(1, 4):
                    nc.vector.scalar_tensor_tensor(
                        acc[:, :], xr_sb[di][:, k:LC + k], convw_t[di][:, k:k + 1],
                        acc[:, :], ALU.mult, ALU.add)
                xct = xc_keep[g][di]
                nc.scalar.activation(xct[:, :], acc[:, :], AF.Silu,
                                     bias=convb_t[di][:, 0:1])
                xc_sb.append(xct)
            xdbl_ps = p1ps.tile([RK + 2 * NST, LC], FP32, tag="xdbl", name="xdbl", bufs=1)
            for di in range(NDT):
                nc.tensor.matmul(xdbl_ps[:, :], xprojT_t[di][:, :], xc_sb[di][:, :],
                                 start=(di == 0), stop=(di == NDT - 1))
            xdbl_sb = p1.tile([RK + 2 * NST, LC], FP32, tag="xdblsb")
            nc.scalar.copy(xdbl_sb[:, :], xdbl_ps[:, :])
            # pack A2A#1 slots
            xdbl_bf = p1.tile([RK + 2 * NST, LC], BF16, tag="xdblbf", name="xdblbf")
            nc.vector.tensor_copy(xdbl_bf[:, :], xdbl_sb[:, :])
            xc_f8 = []
            for di in range(NDT):
                t8 = p1.tile([128, LC], FP8, tag=f"xcf8_{di}", name=f"xcf8_{di}")
                nc.vector.tensor_copy(t8[:, :], xc_keep[g][di][:, :])
                xc_f8.append(t8)
            for tgt in range(NC):
                for j, (qg, qdh, qn0) in enumerate(q_table[tgt]):
                    if qg != g:
                        continue
                    ti = 0 if j == 0 else 1
                    r0 = 0 if j == 0 else (j - 1) * 80
                    tt = a2a_in[ti]
                    w1 = nc.sync.dma_start(out=tt[tgt, r0:r0 + 8, :],
                                           in_=xdbl_bf[0:RK, :])
                    xrow = tt[tgt, r0 + 8:r0 + 72, :].bitcast(FP8)
                    w2 = nc.sync.dma_start(out=bass.AP(tensor=xrow.tensor,
                                                       offset=xrow.offset,
                                                       ap=[list(xrow.ap[0]), [1, LC]]),
                                           in_=xc_f8[qdh][0:64, :])
                    w2b = nc.sync.dma_start(out=bass.AP(tensor=xrow.tensor,
                                                        offset=xrow.offset + LC,
                                                        ap=[list(xrow.ap[0]), [1, LC]]),
                                            in_=xc_f8[qdh][64:128, :])
                    w3 = nc.sync.dma_start(
                        out=tt[tgt, r0 + 72:r0 + 76, :],
                        in_=xdbl_bf[RK + qn0:RK + qn0 + 4, :])
                    w4 = nc.sync.dma_start(
                        out=tt[tgt, r0 + 76:r0 + 80, :],
                        in_=xdbl_bf[RK + NST + qn0:RK + NST + qn0 + 4, :])
                    pack_writes[ti] += [w1, w2, w2b, w3, w4]

        p1_cm.__exit__(None, None, None)
        p1ps_cm.__exit__(None, None, None)

        # ========== A2A#1 (split: quarter 0 first, then quarters 1,2) ====
        cc1s = []
        for ti in range(2):
            cc = nc.gpsimd.collective_compute(
                "AllToAll", ALU.bypass, replica_groups=[list(range(NC))],
                ins=[a2a_in[ti][:, :, :]], outs=[a2a_out[ti][:, :, :]])
            for wi in pack_writes[ti]:
                add_dep_helper(cc.ins, wi.ins, reason="a2a1 after pack writes")
            cc1s.append(cc)

        # ========== P2 ==========
        p2_cm = tc.tile_pool(name="p2", bufs=1)
        p2 = p2_cm.__enter__()
        p2ps_cm = tc.tile_pool(name="p2ps", bufs=1, space="PSUM")
        p2ps = p2ps_cm.__enter__()
        # P2a: delta (stored as l = -delta) for each quarter from dt rows
        l_all = []
        def qsrc(j):
            ti = 0 if j == 0 else 1
            return a2a_out[ti], (0 if j == 0 else (j - 1) * 80), cc1s[ti]
        for j in range(QPC):
            tt, r0, ccj = qsrc(j)
            dt_rows = p2.tile([RK, L], BF16, tag="dtrows", name=f"dtrows{j}")
            srcdt = tt[:, r0:r0 + 8, :].rearrange("r p l -> p r l")
            d0 = nc.sync.dma_start(out=dt_rows[:, :], in_=srcdt)
            add_dep_helper(d0.ins, ccj.ins, reason="read dt after a2a1")
            dpre = p2ps.tile([128, L], FP32, tag="p2big", name=f"dpre{j}", bufs=1)
            for c0 in range(0, L, 512):
                nc.tensor.matmul(dpre[:, c0:c0 + 512],
                                 dtprojTq_sb[:, j * 128:(j + 1) * 128],
                                 dt_rows[:, c0:c0 + 512], start=True, stop=True)
            sg = p2.tile([128, L], BF16, tag="sg2", name=f"sg2{j}")
            nc.scalar.activation(sg[:, :], dpre[:, :], AF.Sigmoid, scale=-1.0,
                                 bias=negdtbq_t[j][:, 0:1])
            lt = p2.tile([128, L], BF16, tag=f"lrows{j}", name=f"lrows{j}")
            nc.scalar.activation(lt[:, :], sg[:, :], AF.Ln)
            l_all.append(lt)
        for j in range(QPC):
            tt, r0, ccj = qsrc(j)
            l_rows = l_all[j]
            xc_rows = p2.tile([128, L], FP8, tag="xcrows")
            xsrc = tt[:, r0 + 8:r0 + 72, :].bitcast(FP8)
            lo = bass.AP(tensor=xsrc.tensor, offset=xsrc.offset,
                         ap=[list(xsrc.ap[0]), list(xsrc.ap[1]), [1, LC]])
            hi = bass.AP(tensor=xsrc.tensor, offset=xsrc.offset + LC,
                         ap=[list(xsrc.ap[0]), list(xsrc.ap[1]), [1, LC]])
            d2 = nc.sync.dma_start(out=xc_rows[0:64, :],
                                   in_=lo.rearrange("r p l -> p r l"))
            add_dep_helper(d2.ins, ccj.ins, reason="read xclo after a2a1")
            d2b = nc.sync.dma_start(out=xc_rows[64:128, :],
                                    in_=hi.rearrange("r p l -> p r l"))
            add_dep_helper(d2b.ins, ccj.ins, reason="read xchi after a2a1")
            du = p2.tile([128, L], BF16, tag="du")
            nc.vector.scalar_tensor_tensor(du[:, :], l_rows[:, :], -1.0,
                                           xc_rows[:, :], ALU.mult, ALU.mult)
            yps = p2ps.tile([128, L], FP32, tag="p2big", name="yps", bufs=1)
            for nn in range(4):
                dA = p2.tile([128, L], BF16, tag="dA", name="dA", bufs=2)
                nc.scalar.activation(dA[:, :], l_rows[:, :], AF.Exp,
                                     scale=negA_sb[:, j * 4 + nn:j * 4 + nn + 1])
                B_bc = p2.tile([128, L], BF16, tag="Bbc", name="Bbc", bufs=2)
                src = tt[:, r0 + 72 + nn, :]
                srcb = bass.AP(tensor=src.tensor, offset=src.offset,
                               ap=[[0, 128]] + [list(p) for p in src.ap])
                d3 = nc.sync.dma_start(out=B_bc[:, :], in_=srcb)
                add_dep_helper(d3.ins, ccj.ins, reason="bbc after a2a1")
                dBu = p2.tile([128, L], BF16, tag="dBu", name="dBu", bufs=1)
                nc.vector.tensor_tensor(dBu[:, :], du[:, :], B_bc[:, :], ALU.mult)
                h = p2.tile([128, L], BF16, tag="h")
                nc.vector.tensor_tensor_scan(h[:, :], dA[:, :], dBu[:, :], 0.0,
                                             ALU.mult, ALU.add)
                C_bc = p2.tile([128, L], BF16, tag="Cbc", name="Cbc", bufs=2)
                src = tt[:, r0 + 76 + nn, :]
                srcb = bass.AP(tensor=src.tensor, offset=src.offset,
                               ap=[[0, 128]] + [list(p) for p in src.ap])
                d4 = nc.sync.dma_start(out=C_bc[:, :], in_=srcb)
                add_dep_helper(d4.ins, ccj.ins, reason="cbc after a2a1")
                hC = p2.tile([128, L], BF16, tag="hC", name="hC", bufs=2)
                nc.vector.tensor_tensor(hC[:, :], h[:, :], C_bc[:, :], ALU.mult)
                for c0 in range(0, L, 512):
                    cw = min(512, L - c0)
                    nc.tensor.matmul(yps[:, c0:c0 + cw], ident_sb[:, :],
                                     hC[:, c0:c0 + cw], start=(nn == 0),
                                     stop=(nn == 3))
            yb = p2.tile([128, L], FP8, tag="ybf")
            nc.scalar.copy(yb[:, :], yps[:, :])
            dst = ya_in[j][:, :, :].rearrange("r p l -> p r l")
            wv = nc.sync.dma_start(out=dst, in_=yb[:, :])
            cc2j = nc.gpsimd.collective_compute(
                "AllToAll", ALU.bypass, replica_groups=[list(range(NC))],
                ins=[ya_in[j][:, :, :]], outs=[ya_out[j][:, :, :]])
            add_dep_helper(cc2j.ins, wv.ins, reason="a2a2j after y write")
            ya_ccs.append(cc2j)

        p2_cm.__exit__(None, None, None)
        p2ps_cm.__exit__(None, None, None)

        # ========== P3 ==========
        p3 = ctx.enter_context(tc.tile_pool(name="p3", bufs=2))
        p3ps = ctx.enter_context(tc.tile_pool(name="p3ps", bufs=1, space="PSUM"))
        out1_ps = p3ps.tile([C, LC], FP32, tag="out1")
        first = True
        for g in range(3):
            for dh in range(dm.NDH):
                owners = co_own[(g, dh)]
                y_sum = p3.tile([128, LC], FP32, tag="ysum")
                for oi, (s, j) in enumerate(owners):
                    yp = p3.tile([128, LC], FP8, tag="ypart")
                    r3 = nc.sync.dma_start(out=yp[:, :], in_=ya_out[j][s, :, :])
                    add_dep_helper(r3.ins, ya_ccs[j].ins, reason="read y after a2a2")
                    if oi == 0:
                        nc.vector.tensor_copy(y_sum[:, :], yp[:, :])
                    else:
                        nc.vector.tensor_tensor(y_sum[:, :], y_sum[:, :], yp[:, :],
                                                ALU.add)
                yss = p3.tile([128, LC], FP32, tag="yss")
                nc.vector.scalar_tensor_tensor(
                    yss[:, :], xc_keep[g][dh][:, :], Dcol_t[dh][:, 0:1],
                    y_sum[:, :], ALU.mult, ALU.add)
                sz = p3.tile([128, LC], FP32, tag="sz")
                nc.scalar.activation(sz[:, :], z_keep[g][dh][:, :], AF.Silu)
                ym = p3.tile([128, LC], FP32, tag="ym")
                nc.vector.tensor_tensor(ym[:, :], yss[:, :], sz[:, :], ALU.mult)
                nc.tensor.matmul(out1_ps[:, :], Wct[(g, dh)][:, :], ym[:, :],
                                 start=first, stop=(g == 2 and dh == dm.NDH - 1))
                first = False
        xres = p3.tile([C, LC], FP32, tag="xres")
        nc.sync.dma_start(out=xres, in_=x_slice[:, :])
        t0 = p3.tile([C, LC], FP32, tag="t0")
        nc.vector.tensor_scalar(t0[:, :], out1_ps[:, :], projb_sb[:, 0:1], None, ALU.add)
        out_res = p3.tile([C, LC], FP32, tag="outres")
        nc.vector.tensor_tensor(out_res[:, :], t0[:, :], xres[:, :], ALU.add)
        xln = part_norm(p3, p3ps, out_res, LC, lnw_sb, lnb_sb, 1e-6, "c")
        gl = []
        for ot in range(4 * C // 128):
            f1 = p3ps.tile([128, LC], FP32, tag="f1ps", name="f1ps", bufs=2)
            nc.tensor.matmul(f1[:, :], fc1T_sb[:, ot * 128:(ot + 1) * 128], xln[:, :],
                             start=True, stop=True)
            gt = p3.tile([128, LC], FP32, tag=f"gelu{ot}")
            nc.scalar.activation(gt[:, :], f1[:, :], AF.Gelu, bias=fc1b_t[ot][:, 0:1])
            gl.append(gt)
        f2 = p3ps.tile([C, LC], FP32, tag="f2ps")
        for ot in range(4 * C // 128):
            nc.tensor.matmul(f2[:, :], fc2T_t[ot][:, :], gl[ot][:, :],
                             start=(ot == 0), stop=(ot == 4 * C // 128 - 1))
        fin = p3.tile([C, LC], FP32, tag="fin")
        nc.vector.tensor_scalar(fin[:, :], f2[:, :], fc2b_sb[:, 0:1], None, ALU.add)
        nc.vector.tensor_tensor(fin[:, :], fin[:, :], out_res[:, :], ALU.add)
        nc.sync.dma_start(out=out_slice[:, :], in_=fin[:, :])

    return nc


def assemble_output(dm: Dims, results):
    C, E, L, LC = dm.C, dm.E, dm.L, dm.LC
    out = np.zeros((C, L), np.float32)
    for c in range(dm.n_cores):
        out[:, c * LC:(c + 1) * LC] = results[c]["out_slice"]
    return out.reshape(1, C, E, E, E)


def make_small_inputs(dm: Dims, seed=0):
    rng = np.random.default_rng(seed)
    C, DI, RK, NST = dm.C, dm.D_INNER, dm.DT_RANK, dm.NST

    def w(shape, s=0.02):
        return (rng.standard_normal(shape) * s).astype(np.float32)

    return {
        "x": rng.standard_normal((1, C, dm.E, dm.E, dm.E)).astype(np.float32),
        "ln_w": np.ones(C, np.float32), "ln_b": np.zeros(C, np.float32),
        "mnorm_w": np.ones(C, np.float32), "mnorm_b": np.zeros(C, np.float32),
        "in_proj_w": w((2 * DI, C)),
        "conv_w": w((DI, 4), 0.2), "conv_b": np.zeros(DI, np.float32),
        "x_proj_w": w((RK + 2 * NST, DI)),
        "dt_proj_w": w((DI, RK), 0.1),
        "dt_proj_b": np.full(DI, float(np.log(np.expm1(0.01))), np.float32),
        "A_log": np.log(np.tile(np.arange(1, NST + 1, dtype=np.float32), (DI, 1))),
        "D_param": np.ones(DI, np.float32),
        "out_proj_w": w((C, DI)),
        "proj_w": w((C, 3 * C)),
        "proj_b": np.zeros(C, np.float32),
        "fc1_w": w((4 * C, C)), "fc1_b": np.zeros(4 * C, np.float32),
        "fc2_w": w((C, 4 * C)), "fc2_b": np.zeros(C, np.float32),
    }


# ============================ kernel entry ============================
_CACHE = {}


def kernel(**inputs):
    """Full-input DFNet kernel on 8 Trainium2 NeuronCores.

    Takes the full (unsharded) inputs as in reference.setup_inputs(), returns
    the full (1, 128, 16, 16, 16) float32 output.
    """
    dm = _CACHE.get("dm")
    if dm is None:
        dm = Dims(E=16)
        _CACHE["dm"] = dm
    nc = _CACHE.get("nc")
    if nc is None:
        nc = build_program(dm)
        _CACHE["nc"] = nc
    in_maps = host_prep(dm, inputs)
    from concourse.bass_utils import run_bass_kernel_spmd
    res = run_bass_kernel_spmd(nc, in_maps, list(range(dm.n_cores)))
    return assemble_output(dm, res.results)

